# revision 4
# baseline (speedup 1.0000x reference)
"""Trainium2 Bass kernel for the KnowledgeCircuit MoE-routing module.

Math (per token t):
    h[t, :]   = sum_n w1[t, n] * (x[t, :] @ F[n])      # D -> R
    out[t, :] = sum_n w2[t, n] * (h[t, :] @ Rst[n])    # R -> D

Sharding: data-parallel over tokens (B*S = 8192 -> 1024 per core across 8
NeuronCores); the [N,D,R]/[N,R,D] knowledge tensors are replicated and
streamed through SBUF once per stage.

Per-core dataflow (tokens on PSUM partitions):
  stage 1: for each expert pair p, token tile t:
      psum[t128, 512] += xT[d,t128].T @ F_pair[d, 512]   (accumulate over d)
      h[t128, 256]  <- scalar_tensor_tensor(psum_half * w1[t, n] + h)
  transpose h once per token tile (PE transpose, 128x128 blocks)
  stage 2: for each expert n, token tile t, d-half:
      psum[t128, 512] += hT[r,t128].T @ Rst[n][r, 512]   (accumulate over r)
      out[t128, 512] <- scalar_tensor_tensor(psum * w2[t, n] + out)
"""

import os
import sys

import numpy as np

for _p in ("/opt/trn_rl_repo", "/root/.axon_site/_ro/trn_rl_repo"):
    if os.path.isdir(_p) and _p not in sys.path:
        sys.path.insert(0, _p)

import ml_dtypes  # noqa: E402

import concourse.bacc as bacc  # noqa: E402
import concourse.bass as bass  # noqa: E402
import concourse.mybir as mybir  # noqa: E402
from concourse.bass_utils import run_bass_kernel_spmd  # noqa: E402
from concourse.masks import make_identity  # noqa: E402
from concourse.tile import TileContext  # noqa: E402

B, S, D, R, N = 4, 2048, 1024, 256, 16
NCORES = 8
T = B * S          # 8192 tokens
TL = T // NCORES   # 1024 tokens per core
TT = TL // 128     # 8 token tiles per core
DT = D // 128      # 8 contraction tiles (stage 1)
RT = R // 128      # 2 contraction tiles (stage 2)
NPAIR = N // 2     # expert pairs sharing one PSUM bank in stage 1

# "f32" (bit-exact-ish), "f32r" (fp32 data, faster reduced-precision PE mode),
# "bf16" (operands cast to bf16, fp32 PSUM accumulation).
MM_DTYPE = os.environ.get("KC_MM_DTYPE", "f32")

LAST_RESULTS = None   # BassKernelResults of the most recent run (for test.py)
_NC_CACHE: dict = {}

_MULT = None
_ADD = None


def _build(mm: str) -> "bass.Bass":
    f32 = mybir.dt.float32
    mdt = mybir.dt.bfloat16 if mm == "bf16" else f32
    rdt = mybir.dt.float32r if mm == "f32r" else None
    MULT = mybir.AluOpType.mult
    ADD = mybir.AluOpType.add

    nc = bacc.Bacc("TRN2")
    xT_d = nc.declare_dram_parameter("xT", [128, DT, TL], mdt, isOutput=False)
    fk_d = nc.declare_dram_parameter("fk", [NPAIR, 128, DT, 2 * R], mdt, isOutput=False)
    rk_d = nc.declare_dram_parameter("rk", [N, 128, RT, D], mdt, isOutput=False)
    w1_d = nc.declare_dram_parameter("w1", [128, TT, N], f32, isOutput=False)
    w2_d = nc.declare_dram_parameter("w2", [128, TT, N], f32, isOutput=False)
    out_d = nc.declare_dram_parameter("out", [128, TT, D], f32, isOutput=True)

    def mmop(ap):
        return ap.bitcast(rdt) if rdt is not None else ap

    with TileContext(nc) as tc:
        with (
            tc.tile_pool(name="const", bufs=1) as const,
            tc.tile_pool(name="fpool", bufs=3) as fpool,
            tc.tile_pool(name="rpool", bufs=3) as rpool,
            tc.tile_pool(name="acc", bufs=1) as acc,
            tc.tile_pool(name="psA", bufs=3, space="PSUM") as psA,
            tc.tile_pool(name="psT", bufs=2, space="PSUM") as psT,
            tc.tile_pool(name="psB", bufs=3, space="PSUM") as psB,
        ):
            xT_sb = const.tile([128, DT, TL], mdt)
            nc.sync.dma_start(out=xT_sb, in_=xT_d[:])
            w1_sb = const.tile([128, TT, N], f32)
            nc.sync.dma_start(out=w1_sb, in_=w1_d[:])
            w2_sb = const.tile([128, TT, N], f32)
            nc.sync.dma_start(out=w2_sb, in_=w2_d[:])
            ident = const.tile([128, 128], f32)
            make_identity(nc, ident)

            h_all = acc.tile([128, TT, R], f32)
            hT_all = acc.tile([128, RT, TT, 128], mdt)
            out_all = acc.tile([128, TT, D], f32)

            # ---------------- stage 1: h = sum_n w1_n * (x @ F_n) -------------
            for p in range(NPAIR):
                f_sb = fpool.tile([128, DT, 2 * R], mdt)
                nc.sync.dma_start(out=f_sb, in_=fk_d[p])
                for t in range(TT):
                    ps = psA.tile([128, 2 * R], mybir.dt.float32)
                    for d in range(DT):
                        nc.tensor.matmul(
                            ps,
                            mmop(xT_sb[:, d, t * 128:(t + 1) * 128]),
                            mmop(f_sb[:, d, :]),
                            start=(d == 0),
                            stop=(d == DT - 1),
                        )
                    for ni in range(2):
                        n = 2 * p + ni
                        ysl = ps[:, ni * R:(ni + 1) * R]
                        wsc = w1_sb[:, t, n:n + 1]
                        hsl = h_all[:, t, :]
                        if n == 0:
                            nc.vector.tensor_scalar_mul(hsl, ysl, wsc)
                        else:
                            nc.vector.scalar_tensor_tensor(
                                out=hsl, in0=ysl, scalar=wsc, in1=hsl,
                                op0=MULT, op1=ADD,
                            )

            # ---------------- transpose h -> hT (and cast to matmul dtype) ----
            for t in range(TT):
                for rt in range(RT):
                    pst = psT.tile([128, 128], mybir.dt.float32)
                    nc.tensor.transpose(
                        pst, h_all[:, t, rt * 128:(rt + 1) * 128], ident
                    )
                    nc.vector.tensor_copy(out=hT_all[:, rt, t, :], in_=pst)

            # ---------------- stage 2: out = sum_n w2_n * (h @ Rst_n) ---------
            for n in range(N):
                r_sb = rpool.tile([128, RT, D], mdt)
                nc.sync.dma_start(out=r_sb, in_=rk_d[n])
                for t in range(TT):
                    for dh in range(2):
                        ps = psB.tile([128, 512], mybir.dt.float32)
                        for rt in range(RT):
                            nc.tensor.matmul(
                                ps,
                                mmop(hT_all[:, rt, t, :]),
                                mmop(r_sb[:, rt, dh * 512:(dh + 1) * 512]),
                                start=(rt == 0),
                                stop=(rt == RT - 1),
                            )
                        osl = out_all[:, t, dh * 512:(dh + 1) * 512]
                        wsc = w2_sb[:, t, n:n + 1]
                        if n == 0:
                            nc.vector.tensor_scalar_mul(osl, ps, wsc)
                        else:
                            nc.vector.scalar_tensor_tensor(
                                out=osl, in0=ps, scalar=wsc, in1=osl,
                                op0=MULT, op1=ADD,
                            )

            for t in range(TT):
                nc.sync.dma_start(out=out_d[:, t, :], in_=out_all[:, t, :])

    nc.finalize()  # Bacc: runs wait-splitting + register allocation passes
    return nc


def _get_nc(mm: str) -> "bass.Bass":
    if mm not in _NC_CACHE:
        _NC_CACHE[mm] = _build(mm)
    return _NC_CACHE[mm]


def _prep_inputs(x, fk, rk, w1, w2, mm: str):
    """Host-side shard + layout prep. Returns per-core input maps."""
    np_mdt = ml_dtypes.bfloat16 if mm == "bf16" else np.float32

    # F: [N,D,R] -> [NPAIR, 128, DT, 2R]  (pair 2p/2p+1 side by side in free)
    fkp = np.ascontiguousarray(
        fk.reshape(NPAIR, 2, DT, 128, R).transpose(0, 3, 2, 1, 4)
        .reshape(NPAIR, 128, DT, 2 * R),
        dtype=np_mdt,
    )
    # Rst: [N,R,D] -> [N, 128, RT, D]
    rkp = np.ascontiguousarray(
        rk.reshape(N, RT, 128, D).transpose(0, 2, 1, 3), dtype=np_mdt
    )

    in_maps = []
    for c in range(NCORES):
        xs = x[c * TL:(c + 1) * TL]  # [TL, D]
        xT = np.ascontiguousarray(
            xs.T.reshape(DT, 128, TL).transpose(1, 0, 2), dtype=np_mdt
        )
        w1s = np.ascontiguousarray(
            w1[c * TL:(c + 1) * TL].reshape(TT, 128, N).transpose(1, 0, 2)
        )
        w2s = np.ascontiguousarray(
            w2[c * TL:(c + 1) * TL].reshape(TT, 128, N).transpose(1, 0, 2)
        )
        in_maps.append({"xT": xT, "fk": fkp, "rk": rkp, "w1": w1s, "w2": w2s})
    return in_maps


def _gather(results) -> np.ndarray:
    outs = [
        np.asarray(results[c]["out"]).transpose(1, 0, 2).reshape(TL, D)
        for c in range(NCORES)
    ]
    return np.concatenate(outs, axis=0).reshape(B, S, D).astype(np.float32)


def kernel(**inputs) -> np.ndarray:
    global LAST_RESULTS
    x = np.ascontiguousarray(
        np.asarray(inputs["x"], dtype=np.float32).reshape(T, D)
    )
    fk = np.asarray(inputs["feature_know"], dtype=np.float32)
    rk = np.asarray(inputs["restore_know"], dtype=np.float32)
    w1 = np.asarray(inputs["feature_know_w"], dtype=np.float32).reshape(T, N)
    w2 = np.asarray(inputs["restore_know_w"], dtype=np.float32).reshape(T, N)

    mm = MM_DTYPE
    in_maps = _prep_inputs(x, fk, rk, w1, w2, mm)
    nc = _get_nc(mm)
    res = run_bass_kernel_spmd(nc, in_maps, core_ids=list(range(NCORES)))
    LAST_RESULTS = res
    return _gather(res.results)
